# revision 1
# baseline (speedup 1.0000x reference)
"""BERT self-attention (BS=4, SEQ=2048, HID=768, NH=12) on 8 NeuronCores.

Sharding: core c -> batch b = c//2, head-group g = c%2 (6 heads each).
Per core the Bass kernel computes, for its batch element and 6 heads:
  Q^T/K^T = (Wh @ X^T + b)   in [d, q] layout  (d on partitions)
  V       = X @ Wv^T + bv    in [k, d] layout, rows scaled by mask m[k]
  S^T     = K^T.T-free matmul -> [k_block, q] scores in PSUM
  P^T     = exp(S^T / 8)     (ACT, PSUM->SBUF; mask folded into V)
  ctx^T   = V'.T @ P^T accumulated over k blocks, with a 65th row = mask
            column giving the softmax denominator.
  out     = ctx^T[0:64] * broadcast(1/denom)  -> [64, q] per head
Host does input transposes (free), sharding, and the final [d,q]->[q,d]
untranspose + concat.

Biases are folded in via an appended ones-row on X^T (contraction 769).
"""

import numpy as np

import concourse.bass as bass
import concourse.tile as tile
from concourse import bacc
from concourse import mybir
from concourse.bass_utils import run_bass_kernel_spmd

F32 = mybir.dt.float32
F32R = mybir.dt.float32r
F16 = mybir.dt.float16
DT_MM = F16          # dtype for matmul operands (A/B: F16 vs F32R)
DT_NP = np.float16   # matching numpy dtype for host-side input prep

BS, SEQ, HID, NH, HD = 4, 2048, 768, 12, 64
NCORES = 8
HPC = 6          # heads per core
FCH = 6          # 128-row chunks of the 768 contraction dim
DSH = HPC * HD   # 384 output features per core


def _body(tc, xt_d, wq_d, wk_d, wv_d, mt_d, ot_d):
    nc = tc.nc
    Exp = mybir.ActivationFunctionType.Exp

    with tc.tile_pool(name="persist", bufs=1) as persist:
        # Warm the exp table set ASAP (overlaps the input DMAs).
        dummy = persist.tile([1, 1], F32, tag="dummy")
        nc.vector.memset(dummy, 0.0)
        nc.scalar.activation(out=dummy, in_=dummy, func=Exp)

        mtile = persist.tile([128, 16], DT_MM, tag="mtile")
        nc.sync.dma_start(out=mtile, in_=mt_d[:, :])
        mtf = persist.tile([128, 16], F32, tag="mtf")
        nc.vector.tensor_copy(out=mtf, in_=mtile)
        qt = [persist.tile([128, SEQ], DT_MM, tag=f"qt{j}", name=f"qt{j}") for j in range(3)]
        kt = [persist.tile([128, SEQ], DT_MM, tag=f"kt{j}", name=f"kt{j}") for j in range(3)]
        vt = persist.tile([128, 16, DSH], DT_MM, tag="vt")

        # ---------------- Phase 1: QKV projections ----------------
        with tc.tile_pool(name="xw", bufs=1) as xw, \
             tc.tile_pool(name="qkp", bufs=3, space="PSUM") as qkp, \
             tc.tile_pool(name="vp", bufs=2, space="PSUM") as vp:
            xts = []
            for f in range(FCH):
                t = xw.tile([128, SEQ], DT_MM, tag=f"x{f}")
                nc.sync.dma_start(out=t, in_=xt_d[f * 128:(f + 1) * 128, :])
                xts.append(t)
            xt1 = persist.tile([1, SEQ], DT_MM, tag="x6")
            nc.sync.dma_start(out=xt1, in_=xt_d[768:769, :])

            wmap = {}
            for dram, nm in ((wq_d, "q"), (wk_d, "k"), (wv_d, "v")):
                lst = []
                for f in range(FCH):
                    t = xw.tile([128, DSH], DT_MM, tag=f"w{nm}{f}")
                    nc.sync.dma_start(out=t, in_=dram[f * 128:(f + 1) * 128, :])
                    lst.append(t)
                b = xw.tile([1, DSH], DT_MM, tag=f"w{nm}b")
                nc.sync.dma_start(out=b, in_=dram[768:769, :])
                lst.append(b)
                wmap[nm] = lst

            # Q^T, K^T: [384, 2048] as 3 tiles of [128, 2048]
            for nm, dst in (("q", qt), ("k", kt)):
                wt = wmap[nm]
                for j in range(3):
                    js = slice(j * 128, (j + 1) * 128)
                    for qc in range(4):
                        qs = slice(qc * 512, (qc + 1) * 512)
                        ps = qkp.tile([128, 512], F32, tag="qk")
                        for f in range(FCH):
                            nc.tensor.matmul(ps, lhsT=wt[f][:, js],
                                             rhs=xts[f][:, qs],
                                             start=(f == 0), stop=False)
                        nc.tensor.matmul(ps, lhsT=wt[6][:, js],
                                         rhs=xt1[:, qs],
                                         start=False, stop=True)
                        nc.scalar.copy(out=dst[j][:, qs], in_=ps)

            # V: [2048, 384] as 16 k-blocks, mask-scaled rows
            wt = wmap["v"]
            for kb in range(16):
                ks = slice(kb * 128, (kb + 1) * 128)
                ps = vp.tile([128, DSH], F32, tag="v")
                for f in range(FCH):
                    nc.tensor.matmul(ps, lhsT=xts[f][:, ks], rhs=wt[f],
                                     start=(f == 0), stop=False)
                nc.tensor.matmul(ps, lhsT=xt1[:, ks], rhs=wt[6],
                                 start=False, stop=True)
                nc.vector.tensor_scalar_mul(
                    out=vt[:, kb, :], in0=ps,
                    scalar1=mtf[:, kb:kb + 1])

        # ---------------- Phase 2: attention ----------------
        # PSUM: scores 2x[128,1024] (4 banks) + ctx pair [128,1024] (2) +
        # denoms [97,1024] (2) = 8 banks.
        with tc.tile_pool(name="sp", bufs=2, space="PSUM") as sp, \
             tc.tile_pool(name="cp", bufs=1, space="PSUM") as cp, \
             tc.tile_pool(name="dp", bufs=1, space="PSUM") as dp, \
             tc.tile_pool(name="pp", bufs=3) as pp, \
             tc.tile_pool(name="ctp", bufs=4) as ctp, \
             tc.tile_pool(name="rdp", bufs=4) as rdp, \
             tc.tile_pool(name="osp", bufs=3) as osp:
            for j in range(3):
                heads = (2 * j, 2 * j + 1)
                ostage = {h: osp.tile([64, SEQ], F32, tag="os", name=f"os{h}")
                          for h in heads}
                for qh in range(2):
                    q0 = qh * 1024
                    cab = cp.tile([128, 1024], F32, tag="c", name="cab")
                    dn = dp.tile([97, 1024], F32, tag="d", name="dn")
                    for kb in range(16):
                        ks = slice(kb * 128, (kb + 1) * 128)
                        sab = [sp.tile([128, 1024], F32, tag="s", name="sab")
                               for _ in range(2)]
                        # scores: 2-head row-packed pairs (K=64 at rows 0/64)
                        for qq in range(2):
                            qs = slice(q0 + qq * 512, q0 + (qq + 1) * 512)
                            osl = slice(qq * 512, (qq + 1) * 512)
                            for i in range(2):
                                rows = slice(64 * i, 64 * (i + 1))
                                nc.tensor.matmul(sab[i][:, osl],
                                                 lhsT=kt[j][rows, ks],
                                                 rhs=qt[j][rows, qs],
                                                 start=True, stop=True)
                        pab = []
                        for i in range(2):
                            p = pp.tile([128, 1024], DT_MM, tag="p", name="ptile")
                            nc.scalar.activation(out=p, in_=sab[i], func=Exp,
                                                 scale=0.125)
                            pab.append(p)
                        st, sp_ = (kb == 0), (kb == 15)
                        # ctx: col-packed pair (head A -> out rows 0-63,
                        # head B -> rows 64-127 of the same PSUM tile)
                        for qq in range(2):
                            osl = slice(qq * 512, (qq + 1) * 512)
                            for i in range(2):
                                nc.tensor.matmul(
                                    cab[64 * i:64 * (i + 1), osl],
                                    lhsT=vt[:, kb, heads[i] * 64:(heads[i] + 1) * 64],
                                    rhs=pab[i][:, osl], start=st, stop=sp_,
                                    skip_group_check=True)
                        # denominators: 4-way col-packed m=1 matmuls
                        # rows 0/32 = heads A/B cols 0:512; 64/96 = cols 512:1024
                        for idx, (i, qq) in enumerate(((0, 0), (1, 0), (0, 1), (1, 1))):
                            osl = slice(qq * 512, (qq + 1) * 512)
                            r = 32 * idx
                            nc.tensor.matmul(dn[r:r + 1, osl],
                                             lhsT=mtile[:, kb:kb + 1],
                                             rhs=pab[i][:, osl],
                                             start=st, stop=sp_,
                                             tile_position=(0, r),
                                             skip_group_check=True)
                    # drain: out = ctx / denom
                    for i in range(2):
                        h = heads[i]
                        ct = ctp.tile([64, 1024], F32, tag="ct")
                        nc.vector.tensor_copy(out=ct, in_=cab[64 * i:64 * (i + 1), :])
                        rd = rdp.tile([1, 1024], DT_MM, tag="rd")
                        nc.vector.tensor_copy(out=rd[:, 0:512],
                                              in_=dn[32 * i:32 * i + 1, 0:512])
                        nc.vector.tensor_copy(out=rd[:, 512:1024],
                                              in_=dn[64 + 32 * i:64 + 32 * i + 1, 512:1024])
                        bc = sp.tile([64, 1024], F32, tag="s")
                        for qq in range(2):
                            osl = slice(qq * 512, (qq + 1) * 512)
                            nc.tensor.matmul(bc[:, osl], lhsT=xt1[:, 0:64],
                                             rhs=rd[:, osl],
                                             start=True, stop=True)
                        rcp = ctp.tile([64, 1024], F32, tag="rcp")
                        nc.vector.reciprocal(rcp, bc)
                        nc.vector.tensor_mul(out=ostage[h][:, q0:q0 + 1024],
                                             in0=ct, in1=rcp)
                for h in heads:
                    nc.sync.dma_start(out=ot_d[h], in_=ostage[h])


def build_nc():
    nc = bacc.Bacc("TRN2")
    xt_d = nc.declare_dram_parameter("xt", [HID + 1, SEQ], DT_MM, isOutput=False)
    wq_d = nc.declare_dram_parameter("wqT", [HID + 1, DSH], DT_MM, isOutput=False)
    wk_d = nc.declare_dram_parameter("wkT", [HID + 1, DSH], DT_MM, isOutput=False)
    wv_d = nc.declare_dram_parameter("wvT", [HID + 1, DSH], DT_MM, isOutput=False)
    mt_d = nc.declare_dram_parameter("mt", [128, 16], DT_MM, isOutput=False)
    ot_d = nc.declare_dram_parameter("OT", [HPC, HD, SEQ], F32, isOutput=True)
    with tile.TileContext(nc) as tc:
        _body(tc, xt_d, wq_d, wk_d, wv_d, mt_d, ot_d)
    nc.finalize()
    return nc


_NC_CACHE = None


def _get_nc():
    global _NC_CACHE
    if _NC_CACHE is None:
        _NC_CACHE = build_nc()
    return _NC_CACHE


def make_in_maps(hidden_states, attention_mask, Wq, bq, Wk, bk, Wv, bv):
    in_maps = []
    for c in range(NCORES):
        b, g = c // 2, c % 2
        hs = slice(g * DSH, (g + 1) * DSH)
        xt = np.empty((HID + 1, SEQ), DT_NP)
        xt[:HID] = hidden_states[b].T
        xt[HID] = 1.0
        m = (attention_mask[b, 0, 0] > -1).astype(DT_NP)
        mt = np.ascontiguousarray(m.reshape(16, 128).T)

        def aug(W, bias):
            wa = np.empty((HID + 1, DSH), DT_NP)
            wa[:HID] = W[hs, :].T
            wa[HID] = bias[hs]
            return wa

        in_maps.append({
            "xt": np.ascontiguousarray(xt),
            "wqT": aug(Wq, bq),
            "wkT": aug(Wk, bk),
            "wvT": aug(Wv, bv),
            "mt": mt,
        })
    return in_maps


def gather_out(results):
    out = np.empty((BS, SEQ, HID), np.float32)
    for c in range(NCORES):
        b, g = c // 2, c % 2
        ot = results[c]["OT"]  # [6, 64, 2048]
        out[b, :, g * DSH:(g + 1) * DSH] = (
            ot.transpose(2, 0, 1).reshape(SEQ, DSH)
        )
    return out


def kernel(hidden_states, attention_mask, Wq, bq, Wk, bk, Wv, bv):
    nc = _get_nc()
    in_maps = make_in_maps(hidden_states, attention_mask,
                           Wq, bq, Wk, bk, Wv, bv)
    res = run_bass_kernel_spmd(nc, in_maps, core_ids=list(range(NCORES)))
    return gather_out(res.results)



# revision 5
# speedup vs baseline: 1.8269x; 1.8269x over previous
"""BERT self-attention (BS=4, SEQ=2048, HID=768, NH=12) on 8 NeuronCores.

Sharding: core c -> batch b = c//2, head-group g = c%2 (6 heads each, as
3 head-pairs j=0..2).

Per-core pipeline (all matmuls fp16, PE fully packed):
  V    = X @ Wv^T + bv              [k,d] layout, 16 k-blocks
  K^T j = Wk_j @ X^T (+bias via ACT Identity copy)   [128d, 2048q]
  Q^T j = Wq_j @ X^T (+bias)                          [128d, 2048q]
  attention per (j, qh in 0..3), q-chunk = 512:
    per kb (16 k-blocks of 128):
      scores: 2 row-tiled MMs (c=64 heads packed at row 0/64) ->
              sab [128k, 1024] f32 PSUM (head A cols 0:512, B 512:1024)
      P = exp(S/8 + mask_bias): even kb -> ACT exp (fp16 out),
          odd kb  -> DVE Schraudolph (tensor_scalar -> int16, bitcast fp16)
      ctx: 2 col-tiled MMs accumulate cab[0:64]/[64:128] over kb
      denom: per kb-pair, 4 col-packed m=1 MMs (ones lhsT) into dn rows
             0/32 (A/B even kb) and 64/96 (A/B odd kb)
    DMA cab [128,512] f32 and 4 dn rows -> DRAM
Host: shard/transposes, final out = ctx / (dn_even + dn_odd), concat.
"""

import numpy as np

import concourse.bass as bass
import concourse.tile as tile
from concourse import bacc
from concourse import mybir
from concourse.bass_utils import run_bass_kernel_spmd

F32 = mybir.dt.float32
F16 = mybir.dt.float16
I16 = mybir.dt.int16
DT_NP = np.float16

BS, SEQ, HID, NH, HD = 4, 2048, 768, 12, 64
NCORES = 8
HPC = 6          # heads per core
FCH = 6          # 128-row chunks of the 768 contraction dim
DSH = HPC * HD   # 384 output features per core
NQH = 4          # q-chunks of 512
QCH = 512

LOG2E = float(np.log2(np.e))
SCH_S1 = 1024.0 * 0.125 * LOG2E          # Schraudolph multiplier
SCH_C = -59.0                             # Schraudolph bias correction
SCH_S2 = 15.0 * 1024.0 + SCH_C           # valid-row add constant
SCH_MASKED = -1.0e6                       # masked-row add (saturates -> -0.0)
ACT_MASKED = -30.0                        # masked exp bias (exp -> 0 in fp16)


def _body(tc, xt_d, wq_d, wk_d, wv_d, bcol_d, mcol_d, scol_d, ot_d, dn_d):
    nc = tc.nc
    Exp = mybir.ActivationFunctionType.Exp
    Ident = mybir.ActivationFunctionType.Identity

    with tc.tile_pool(name="persist", bufs=1) as persist, \
         tc.tile_pool(name="sabp", bufs=2, space="PSUM") as sabp, \
         tc.tile_pool(name="cabp", bufs=2, space="PSUM") as cabp, \
         tc.tile_pool(name="dnp", bufs=2, space="PSUM") as dnp, \
         tc.tile_pool(name="pp", bufs=4) as pp:
        # Warm the exp table set ASAP (overlaps the input DMAs).
        dummy = persist.tile([1, 1], F32, tag="dummy")
        nc.vector.memset(dummy, 0.0)
        nc.scalar.activation(out=dummy, in_=dummy, func=Exp)

        ones = persist.tile([128, 1], F16, tag="ones")
        nc.vector.memset(ones, 1.0)

        bcol = persist.tile([128, 6], F32, tag="bcol")
        mcol = persist.tile([128, 16], F32, tag="mcol")
        scol = persist.tile([128, 16], F32, tag="scol")
        nc.sync.dma_start(out=bcol, in_=bcol_d[:, :])
        nc.sync.dma_start(out=mcol, in_=mcol_d[:, :])
        nc.sync.dma_start(out=scol, in_=scol_d[:, :])

        # Input DMAs: interleave wv/x chunks so V matmuls can start early.
        xts, wvl = [], []
        for f in range(FCH):
            w = persist.tile([128, DSH], F16, tag=f"wv{f}")
            nc.sync.dma_start(out=w, in_=wv_d[f * 128:(f + 1) * 128, :])
            wvl.append(w)
            t = persist.tile([128, SEQ], F16, tag=f"x{f}")
            nc.sync.dma_start(out=t, in_=xt_d[f * 128:(f + 1) * 128, :])
            xts.append(t)
        xt1 = persist.tile([1, SEQ], F16, tag="x6")
        nc.sync.dma_start(out=xt1, in_=xt_d[768:769, :])
        wvb = persist.tile([1, DSH], F16, tag="wvb")
        nc.sync.dma_start(out=wvb, in_=wv_d[768:769, :])
        wql, wkl = [], []
        for dram, lst, nm in ((wq_d, wql, "q"), (wk_d, wkl, "k")):
            for f in range(FCH):
                w = persist.tile([128, DSH], F16, tag=f"w{nm}{f}")
                nc.sync.dma_start(out=w, in_=dram[f * 128:(f + 1) * 128, :])
                lst.append(w)

        qt = [persist.tile([128, SEQ], F16, tag=f"qt{j}", name=f"qt{j}") for j in range(3)]
        kt = [persist.tile([128, SEQ], F16, tag=f"kt{j}", name=f"kt{j}") for j in range(3)]
        vt = persist.tile([128, 16, DSH], F16, tag="vt")

        # ---------------- V projection ----------------
        for kb in range(16):
            ks = slice(kb * 128, (kb + 1) * 128)
            ps = sabp.tile([128, 1024], F32, tag="sab", name="vps")
            for f in range(FCH):
                nc.tensor.matmul(ps[:, 0:DSH], lhsT=xts[f][:, ks], rhs=wvl[f],
                                 start=(f == 0), stop=False)
            nc.tensor.matmul(ps[:, 0:DSH], lhsT=xt1[:, ks], rhs=wvb,
                             start=False, stop=True)
            nc.vector.tensor_copy(out=vt[:, kb, :], in_=ps[:, 0:DSH])

        # ---------------- per head-pair: K/Q projection then attention ----
        for j in range(3):
            js = slice(j * 128, (j + 1) * 128)
            for wl, dst, bc in ((wkl, kt[j], 3 + j), (wql, qt[j], j)):
                for qc in range(4):
                    qs = slice(qc * 512, (qc + 1) * 512)
                    ps = sabp.tile([128, 1024], F32, tag="sab", name="qkps")
                    for f in range(FCH):
                        nc.tensor.matmul(ps[:, 0:512], lhsT=wl[f][:, js],
                                         rhs=xts[f][:, qs],
                                         start=(f == 0), stop=(f == FCH - 1))
                    nc.scalar.activation(out=dst[:, qs], in_=ps[:, 0:512],
                                         func=Ident, bias=bcol[:, bc:bc + 1],
                                         scale=1.0)

            hA, hB = 2 * j, 2 * j + 1
            for qh in range(NQH):
                qs = slice(qh * QCH, (qh + 1) * QCH)
                cab = cabp.tile([128, QCH], F32, tag="cab", name="cab")
                dnt = dnp.tile([128, QCH], F32, tag="dn", name="dn")
                pprev = None
                for kb in range(16):
                    ks = slice(kb * 128, (kb + 1) * 128)
                    sab = sabp.tile([128, 1024], F32, tag="sab", name="sab")
                    nc.tensor.matmul(sab[:, 0:512], lhsT=kt[j][0:64, ks],
                                     rhs=qt[j][0:64, qs],
                                     start=True, stop=True)
                    nc.tensor.matmul(sab[:, 512:1024], lhsT=kt[j][64:128, ks],
                                     rhs=qt[j][64:128, qs],
                                     start=True, stop=True)
                    p = pp.tile([128, 1024], F16, tag="p", name="ptile")
                    if kb % 2 == 0:
                        nc.scalar.activation(out=p, in_=sab, func=Exp,
                                             scale=0.125,
                                             bias=mcol[:, kb:kb + 1])
                    else:
                        nc.vector.tensor_scalar(
                            out=p.bitcast(I16), in0=sab,
                            scalar1=SCH_S1, scalar2=scol[:, kb:kb + 1],
                            op0=mybir.AluOpType.mult, op1=mybir.AluOpType.add)
                    st, sp_ = (kb == 0), (kb == 15)
                    nc.tensor.matmul(cab[0:64, :],
                                     lhsT=vt[:, kb, hA * 64:hA * 64 + 64],
                                     rhs=p[:, 0:512], start=st, stop=sp_,
                                     skip_group_check=True)
                    nc.tensor.matmul(cab[64:128, :],
                                     lhsT=vt[:, kb, hB * 64:hB * 64 + 64],
                                     rhs=p[:, 512:1024], start=st, stop=sp_,
                                     skip_group_check=True)
                    if kb % 2 == 1:
                        st2, sp2 = (kb == 1), (kb == 15)
                        for r, rhs in ((0, pprev[:, 0:512]),
                                       (32, pprev[:, 512:1024]),
                                       (64, p[:, 0:512]),
                                       (96, p[:, 512:1024])):
                            nc.tensor.matmul(dnt[r:r + 1, :], lhsT=ones,
                                             rhs=rhs, start=st2, stop=sp2,
                                             tile_position=(0, r),
                                             skip_group_check=True)
                    pprev = p
                cstage = pp.tile([128, QCH], F16, tag="cst", name="cstage")
                nc.scalar.copy(out=cstage, in_=cab)
                nc.sync.dma_start(out=ot_d[j, qh], in_=cstage)
                dstage = pp.tile([97, QCH], F32, tag="dstage", name="dstage")
                nc.vector.tensor_copy(out=dstage, in_=dnt[0:97, :])
                nc.sync.dma_start(out=dn_d[j, qh], in_=dstage)


def build_nc():
    nc = bacc.Bacc("TRN2")
    xt_d = nc.declare_dram_parameter("xt", [HID + 1, SEQ], F16, isOutput=False)
    wq_d = nc.declare_dram_parameter("wqT", [HID + 1, DSH], F16, isOutput=False)
    wk_d = nc.declare_dram_parameter("wkT", [HID + 1, DSH], F16, isOutput=False)
    wv_d = nc.declare_dram_parameter("wvT", [HID + 1, DSH], F16, isOutput=False)
    bcol_d = nc.declare_dram_parameter("bcol", [128, 6], F32, isOutput=False)
    mcol_d = nc.declare_dram_parameter("mcol", [128, 16], F32, isOutput=False)
    scol_d = nc.declare_dram_parameter("scol", [128, 16], F32, isOutput=False)
    ot_d = nc.declare_dram_parameter("OT", [3, NQH, 128, QCH], F16,
                                     isOutput=True)
    dn_d = nc.declare_dram_parameter("DN", [3, NQH, 97, QCH], F32,
                                     isOutput=True)
    with tile.TileContext(nc) as tc:
        _body(tc, xt_d, wq_d, wk_d, wv_d, bcol_d, mcol_d, scol_d, ot_d, dn_d)
    nc.finalize()
    return nc


_NC_CACHE = None


def _get_nc():
    global _NC_CACHE
    if _NC_CACHE is None:
        _NC_CACHE = build_nc()
    return _NC_CACHE


def make_in_maps(hidden_states, attention_mask, Wq, bq, Wk, bk, Wv, bv):
    in_maps = []
    for c in range(NCORES):
        b, g = c // 2, c % 2
        hs = slice(g * DSH, (g + 1) * DSH)
        xt = np.empty((HID + 1, SEQ), DT_NP)
        xt[:HID] = hidden_states[b].T
        xt[HID] = 1.0
        keep = np.asarray(attention_mask[b, 0, 0]) > -1       # [SEQ]
        keep_kb = keep.reshape(16, 128).T                     # [128, 16]
        mcol = np.where(keep_kb, 0.0, ACT_MASKED).astype(np.float32)
        scol = np.where(keep_kb, SCH_S2, SCH_MASKED).astype(np.float32)
        # Q bias cols 0..2, K bias cols 3..5 (per 128-d j-tile)
        bcol = np.empty((128, 6), np.float32)
        for j in range(3):
            bcol[:, j] = bq[hs][j * 128:(j + 1) * 128]
            bcol[:, 3 + j] = bk[hs][j * 128:(j + 1) * 128]

        def aug(W, bias):
            wa = np.empty((HID + 1, DSH), DT_NP)
            wa[:HID] = W[hs, :].T
            wa[HID] = bias[hs]
            return wa

        in_maps.append({
            "xt": np.ascontiguousarray(xt),
            "wqT": aug(Wq, bq),
            "wkT": aug(Wk, bk),
            "wvT": aug(Wv, bv),
            "bcol": bcol,
            "mcol": np.ascontiguousarray(mcol),
            "scol": np.ascontiguousarray(scol),
        })
    return in_maps


def gather_out(results):
    out = np.empty((BS, SEQ, HID), np.float32)
    for c in range(NCORES):
        b, g = c // 2, c % 2
        ot = results[c]["OT"].astype(np.float32)   # [3, 4, 128, 512]
        dn = results[c]["DN"]                      # [3, 4, 97, 512] f32
        for j in range(3):
            den_a = (dn[j, :, 0, :] + dn[j, :, 64, :]).reshape(SEQ)  # [2048]
            den_b = (dn[j, :, 32, :] + dn[j, :, 96, :]).reshape(SEQ)
            # ctx rows 0:64 = head 2j, 64:128 = head 2j+1; [4,64,512]->[2048,64]
            ctx_a = ot[j, :, 0:64, :].transpose(0, 2, 1).reshape(SEQ, HD)
            ctx_b = ot[j, :, 64:128, :].transpose(0, 2, 1).reshape(SEQ, HD)
            c0 = g * DSH + (2 * j) * HD
            out[b, :, c0:c0 + HD] = ctx_a / den_a[:, None]
            out[b, :, c0 + HD:c0 + 2 * HD] = ctx_b / den_b[:, None]
    return out


def kernel(hidden_states, attention_mask, Wq, bq, Wk, bk, Wv, bv):
    nc = _get_nc()
    in_maps = make_in_maps(hidden_states, attention_mask,
                           Wq, bq, Wk, bk, Wv, bv)
    res = run_bass_kernel_spmd(nc, in_maps, core_ids=list(range(NCORES)))
    return gather_out(res.results)


# revision 7
# speedup vs baseline: 2.4046x; 1.3162x over previous
"""BERT self-attention (BS=4, SEQ=2048, HID=768, NH=12) on 8 NeuronCores.

Sharding: core c -> batch b = c//2, head-group g = c%2 (6 heads each, as
3 head-pairs j=0..2).

Per-core pipeline (all matmuls fp16, PE fully packed):
  V    = X @ Wv^T + bv              [k,d] layout, 16 k-blocks
  K^T j = Wk_j @ X^T (+bias via ACT Identity copy)   [128d, 2048q]
  Q^T j = Wq_j @ X^T (+bias)                          [128d, 2048q]
  attention per (j, qh in 0..3), q-chunk = 512:
    per kb (16 k-blocks of 128):
      scores: 2 row-tiled MMs (c=64 heads packed at row 0/64) ->
              sab [128k, 1024] f32 PSUM (head A cols 0:512, B 512:1024)
      P = exp(S/8 + mask_bias): even kb -> ACT exp (fp16 out),
          odd kb  -> DVE Schraudolph (tensor_scalar -> int16, bitcast fp16)
      ctx: 2 col-tiled MMs accumulate cab[0:64]/[64:128] over kb
      denom: per kb-pair, 4 col-packed m=1 MMs (ones lhsT) into dn rows
             0/32 (A/B even kb) and 64/96 (A/B odd kb)
    DMA cab [128,512] f32 and 4 dn rows -> DRAM
Host: shard/transposes, final out = ctx / (dn_even + dn_odd), concat.
"""

import numpy as np

import concourse.bass as bass
import concourse.tile as tile
from concourse import bacc
from concourse import mybir
from concourse.bass_utils import run_bass_kernel_spmd

F32 = mybir.dt.float32
F16 = mybir.dt.float16
I16 = mybir.dt.int16
DT_NP = np.float16

BS, SEQ, HID, NH, HD = 4, 2048, 768, 12, 64
NCORES = 8
HPC = 6          # heads per core
FCH = 6          # 128-row chunks of the 768 contraction dim
DSH = HPC * HD   # 384 output features per core
NQH = 4          # q-chunks of 512
QCH = 512

LOG2E = float(np.log2(np.e))
SCH_S1 = 1024.0 * 0.125 * LOG2E          # Schraudolph multiplier
SCH_C = -59.0                             # Schraudolph bias correction
SCH_S2 = 15.0 * 1024.0 + SCH_C           # valid-row add constant
SCH_MASKED = -1.0e6                       # masked-row add (saturates -> -0.0)
ACT_MASKED = -30.0                        # masked exp bias (exp -> 0 in fp16)


def _body(tc, xt_d, wq_d, wk_d, wv_d, bcol_d, mcol_d, scol_d, ot_d, dn_d):
    nc = tc.nc
    Exp = mybir.ActivationFunctionType.Exp
    Ident = mybir.ActivationFunctionType.Identity

    with tc.tile_pool(name="persist", bufs=1) as persist, \
         tc.tile_pool(name="sabp", bufs=2, space="PSUM") as sabp, \
         tc.tile_pool(name="cabp", bufs=2, space="PSUM") as cabp, \
         tc.tile_pool(name="dnp", bufs=2, space="PSUM") as dnp, \
         tc.tile_pool(name="pp", bufs=6) as pp, \
         tc.tile_pool(name="stg", bufs=2) as stg:
        # Warm the exp table set ASAP (overlaps the input DMAs).
        dummy = persist.tile([1, 1], F32, tag="dummy")
        nc.vector.memset(dummy, 0.0)
        nc.scalar.activation(out=dummy, in_=dummy, func=Exp)

        ones = persist.tile([128, 1], F16, tag="ones")
        nc.vector.memset(ones, 1.0)

        bcol = persist.tile([128, 6], F32, tag="bcol")
        mcol = persist.tile([128, 16], F32, tag="mcol")
        scol = persist.tile([128, 16], F32, tag="scol")
        nc.sync.dma_start(out=bcol, in_=bcol_d[:, :])
        nc.sync.dma_start(out=mcol, in_=mcol_d[:, :])
        nc.sync.dma_start(out=scol, in_=scol_d[:, :])

        # Input DMAs: interleave wv/x chunks so V matmuls can start early.
        xts, wvl = [], []
        for f in range(FCH):
            w = persist.tile([128, DSH], F16, tag=f"wv{f}")
            nc.sync.dma_start(out=w, in_=wv_d[f * 128:(f + 1) * 128, :])
            wvl.append(w)
            t = persist.tile([128, SEQ], F16, tag=f"x{f}")
            nc.sync.dma_start(out=t, in_=xt_d[f * 128:(f + 1) * 128, :])
            xts.append(t)
        xt1 = persist.tile([1, SEQ], F16, tag="x6")
        nc.sync.dma_start(out=xt1, in_=xt_d[768:769, :])
        wvb = persist.tile([1, DSH], F16, tag="wvb")
        nc.sync.dma_start(out=wvb, in_=wv_d[768:769, :])
        wql, wkl = [], []
        for dram, lst, nm in ((wq_d, wql, "q"), (wk_d, wkl, "k")):
            for f in range(FCH):
                w = persist.tile([128, DSH], F16, tag=f"w{nm}{f}")
                nc.sync.dma_start(out=w, in_=dram[f * 128:(f + 1) * 128, :])
                lst.append(w)

        qt = [persist.tile([128, SEQ], F16, tag=f"qt{j}", name=f"qt{j}") for j in range(3)]
        kt = [persist.tile([128, SEQ], F16, tag=f"kt{j}", name=f"kt{j}") for j in range(3)]
        vt = persist.tile([128, 16, DSH], F16, tag="vt")

        # ---------------- V projection ----------------
        for kb in range(16):
            ks = slice(kb * 128, (kb + 1) * 128)
            ps = sabp.tile([128, 1024], F32, tag="sab", name="vps")
            for f in range(FCH):
                nc.tensor.matmul(ps[:, 0:DSH], lhsT=xts[f][:, ks], rhs=wvl[f],
                                 start=(f == 0), stop=False)
            nc.tensor.matmul(ps[:, 0:DSH], lhsT=xt1[:, ks], rhs=wvb,
                             start=False, stop=True)
            nc.vector.tensor_copy(out=vt[:, kb, :], in_=ps[:, 0:DSH])

        # ---------------- per head-pair: K/Q projection then attention ----
        for j in range(3):
            js = slice(j * 128, (j + 1) * 128)
            for wl, dst, bc in ((wkl, kt[j], 3 + j), (wql, qt[j], j)):
                for qc in range(4):
                    qs = slice(qc * 512, (qc + 1) * 512)
                    ps = sabp.tile([128, 1024], F32, tag="sab", name="qkps")
                    for f in range(FCH):
                        nc.tensor.matmul(ps[:, 0:512], lhsT=wl[f][:, js],
                                         rhs=xts[f][:, qs],
                                         start=(f == 0), stop=(f == FCH - 1))
                    nc.scalar.activation(out=dst[:, qs], in_=ps[:, 0:512],
                                         func=Ident, bias=bcol[:, bc:bc + 1],
                                         scale=1.0)

            hA, hB = 2 * j, 2 * j + 1
            for qh in range(NQH):
                qs = slice(qh * QCH, (qh + 1) * QCH)
                cab = cabp.tile([128, QCH], F32, tag="cab", name="cab")
                dnt = dnp.tile([128, QCH], F32, tag="dn", name="dn")
                ptiles = {}

                def emit_ctx(kb):
                    st, sp_ = (kb == 0), (kb == 15)
                    p = ptiles[kb]
                    nc.tensor.matmul(cab[0:64, :],
                                     lhsT=vt[:, kb, hA * 64:hA * 64 + 64],
                                     rhs=p[:, 0:512], start=st, stop=sp_,
                                     skip_group_check=True)
                    nc.tensor.matmul(cab[64:128, :],
                                     lhsT=vt[:, kb, hB * 64:hB * 64 + 64],
                                     rhs=p[:, 512:1024], start=st, stop=sp_,
                                     skip_group_check=True)
                    if kb % 2 == 1:
                        # all four rhs become ready with P(kb) -> the
                        # scheduler issues them as one 4x col-tiled group
                        st2, sp2 = (kb == 1), (kb == 15)
                        pm, pe_ = ptiles[kb], ptiles[kb - 1]
                        for r, rhs in ((0, pm[:, 0:512]),
                                       (32, pm[:, 512:1024]),
                                       (64, pe_[:, 0:512]),
                                       (96, pe_[:, 512:1024])):
                            nc.tensor.matmul(dnt[r:r + 1, :], lhsT=ones,
                                             rhs=rhs, start=st2, stop=sp2,
                                             tile_position=(0, r),
                                             skip_group_check=True)
                        del ptiles[kb - 1], ptiles[kb]

                for kb in range(16):
                    ks = slice(kb * 128, (kb + 1) * 128)
                    sab = sabp.tile([128, 1024], F32, tag="sab", name="sab")
                    nc.tensor.matmul(sab[:, 0:512], lhsT=kt[j][0:64, ks],
                                     rhs=qt[j][0:64, qs],
                                     start=True, stop=True)
                    nc.tensor.matmul(sab[:, 512:1024], lhsT=kt[j][64:128, ks],
                                     rhs=qt[j][64:128, qs],
                                     start=True, stop=True)
                    p = pp.tile([128, 1024], F16, tag="p", name="ptile")
                    if kb % 2 == 0:
                        nc.scalar.activation(out=p, in_=sab, func=Exp,
                                             scale=0.125,
                                             bias=mcol[:, kb:kb + 1])
                    else:
                        nc.vector.tensor_scalar(
                            out=p.bitcast(I16), in0=sab,
                            scalar1=SCH_S1, scalar2=scol[:, kb:kb + 1],
                            op0=mybir.AluOpType.mult, op1=mybir.AluOpType.add)
                    ptiles[kb] = p
                    if kb >= 2:
                        emit_ctx(kb - 2)
                emit_ctx(14)
                emit_ctx(15)
                cstage = stg.tile([128, QCH], F16, tag="cst", name="cstage")
                nc.scalar.copy(out=cstage, in_=cab)
                nc.sync.dma_start(out=ot_d[j, qh], in_=cstage)
                dstage = stg.tile([97, QCH], F32, tag="dstage", name="dstage")
                nc.vector.tensor_copy(out=dstage, in_=dnt[0:97, :])
                nc.sync.dma_start(out=dn_d[j, qh], in_=dstage)


def build_nc():
    nc = bacc.Bacc("TRN2")
    xt_d = nc.declare_dram_parameter("xt", [HID + 1, SEQ], F16, isOutput=False)
    wq_d = nc.declare_dram_parameter("wqT", [HID + 1, DSH], F16, isOutput=False)
    wk_d = nc.declare_dram_parameter("wkT", [HID + 1, DSH], F16, isOutput=False)
    wv_d = nc.declare_dram_parameter("wvT", [HID + 1, DSH], F16, isOutput=False)
    bcol_d = nc.declare_dram_parameter("bcol", [128, 6], F32, isOutput=False)
    mcol_d = nc.declare_dram_parameter("mcol", [128, 16], F32, isOutput=False)
    scol_d = nc.declare_dram_parameter("scol", [128, 16], F32, isOutput=False)
    ot_d = nc.declare_dram_parameter("OT", [3, NQH, 128, QCH], F16,
                                     isOutput=True)
    dn_d = nc.declare_dram_parameter("DN", [3, NQH, 97, QCH], F32,
                                     isOutput=True)
    with tile.TileContext(nc) as tc:
        _body(tc, xt_d, wq_d, wk_d, wv_d, bcol_d, mcol_d, scol_d, ot_d, dn_d)
    nc.finalize()
    return nc


_NC_CACHE = None


def _get_nc():
    global _NC_CACHE
    if _NC_CACHE is None:
        _NC_CACHE = build_nc()
    return _NC_CACHE


def make_in_maps(hidden_states, attention_mask, Wq, bq, Wk, bk, Wv, bv):
    in_maps = []
    for c in range(NCORES):
        b, g = c // 2, c % 2
        hs = slice(g * DSH, (g + 1) * DSH)
        xt = np.empty((HID + 1, SEQ), DT_NP)
        xt[:HID] = hidden_states[b].T
        xt[HID] = 1.0
        keep = np.asarray(attention_mask[b, 0, 0]) > -1       # [SEQ]
        keep_kb = keep.reshape(16, 128).T                     # [128, 16]
        mcol = np.where(keep_kb, 0.0, ACT_MASKED).astype(np.float32)
        scol = np.where(keep_kb, SCH_S2, SCH_MASKED).astype(np.float32)
        # Q bias cols 0..2, K bias cols 3..5 (per 128-d j-tile)
        bcol = np.empty((128, 6), np.float32)
        for j in range(3):
            bcol[:, j] = bq[hs][j * 128:(j + 1) * 128]
            bcol[:, 3 + j] = bk[hs][j * 128:(j + 1) * 128]

        def aug(W, bias):
            wa = np.empty((HID + 1, DSH), DT_NP)
            wa[:HID] = W[hs, :].T
            wa[HID] = bias[hs]
            return wa

        in_maps.append({
            "xt": np.ascontiguousarray(xt),
            "wqT": aug(Wq, bq),
            "wkT": aug(Wk, bk),
            "wvT": aug(Wv, bv),
            "bcol": bcol,
            "mcol": np.ascontiguousarray(mcol),
            "scol": np.ascontiguousarray(scol),
        })
    return in_maps


def gather_out(results):
    out = np.empty((BS, SEQ, HID), np.float32)
    for c in range(NCORES):
        b, g = c // 2, c % 2
        ot = results[c]["OT"].astype(np.float32)   # [3, 4, 128, 512]
        dn = results[c]["DN"]                      # [3, 4, 97, 512] f32
        for j in range(3):
            den_a = (dn[j, :, 0, :] + dn[j, :, 64, :]).reshape(SEQ)  # [2048]
            den_b = (dn[j, :, 32, :] + dn[j, :, 96, :]).reshape(SEQ)
            # ctx rows 0:64 = head 2j, 64:128 = head 2j+1; [4,64,512]->[2048,64]
            ctx_a = ot[j, :, 0:64, :].transpose(0, 2, 1).reshape(SEQ, HD)
            ctx_b = ot[j, :, 64:128, :].transpose(0, 2, 1).reshape(SEQ, HD)
            c0 = g * DSH + (2 * j) * HD
            out[b, :, c0:c0 + HD] = ctx_a / den_a[:, None]
            out[b, :, c0 + HD:c0 + 2 * HD] = ctx_b / den_b[:, None]
    return out


def kernel(hidden_states, attention_mask, Wq, bq, Wk, bk, Wv, bv):
    nc = _get_nc()
    in_maps = make_in_maps(hidden_states, attention_mask,
                           Wq, bq, Wk, bk, Wv, bv)
    res = run_bass_kernel_spmd(nc, in_maps, core_ids=list(range(NCORES)))
    return gather_out(res.results)
